# revision 22
# baseline (speedup 1.0000x reference)
"""Causal multi-head attention on 8 Trainium2 NeuronCores.

v2: (batch x head-group) sharding — core c handles batch b=c//2 and heads
[8g:8g+8] (g=c%2), i.e. 4 row-packed head PAIRS per core. All matmul
operands are bf16 (fp32r streams at half the PE rate; bf16 runs 1 col/cycle
at 2.4GHz once the p-state ramps). PSUM accumulates in f32.

Per-core layout (partition dim first):
  t_x    [128, 8, 2048]  x_b^T tiled: [p, kd, seq], d = kd*128+p
  t_w    [128, 8, 1536]  wqkv tiles: [p, kd, q512|k512|v512]
  qT/kT  [128, 4, 2048]  pair m: partitions = m's 128 head dims
  tv     [128, 16, 512]  per key tile: 8 x v_head(64)
  scores [128, 1024]     psum, both heads of a pair (row-packed matmuls)
  ctx    [128, 512]      psum per pair: h0 rows 0:64, h1 rows 64:128
                         (col-tiled matmuls run both heads concurrently)
  esum   [128, 1024]     bf16 running exp-sums; softmax denominators via
                         gpsimd partition_all_reduce at pair end
  ctxT   [128, 4, 2048]  normalized context, bf16
  out    [2048, 1024]    f32 partial (host sums 2 cores + bo per batch)

V is computed directly in transposed orientation (x-tile stationary,
wv moving) so no on-chip transposes are needed. q/k biases fold into
the psum->SBUF copies (per-partition tensor_scalar_add). Diagonal key
tiles trim scores/exp/ctx to the unmasked query range.

Emission order software-pipelines the attention inner loop (scores kt+1
issued before ctx kt) and weaves projection units for q-chunk qc+1 into
the attention stream of q-chunk qc to keep the tensor engine dense.
"""

import numpy as np
from contextlib import ExitStack

import concourse.bass as bass
import concourse.mybir as mybir
import concourse.tile as tile
from concourse import bacc
from concourse import bass_utils
from concourse.bass import bass_isa

F32 = mybir.dt.float32
BF16 = mybir.dt.bfloat16
AF = mybir.ActivationFunctionType

B, S, D = 4, 2048, 1024
H, DH = 16, 64
NCORES = 8
HG = 512            # head dims per core (8 heads)
NP = 4              # head pairs per core
QC = 512            # q-chunk width
NQC = S // QC       # 4
NKD = D // 128      # 8 contraction tiles
NKT = S // 128      # 16 key tiles

_CACHE = {}


def _build():
    nc = bacc.Bacc("TRN2", target_bir_lowering=False, debug=False)
    xt = nc.dram_tensor("xt", [128, NKD, S], BF16, kind="ExternalInput").ap()
    wqkv = nc.dram_tensor("wqkv", [128, NKD, 3 * HG], BF16, kind="ExternalInput").ap()
    bqk = nc.dram_tensor("bqk", [128, NP, 2], F32, kind="ExternalInput").ap()
    bv = nc.dram_tensor("bv", [1, HG], F32, kind="ExternalInput").ap()
    wo = nc.dram_tensor("wo", [128, NP, D], BF16, kind="ExternalInput").ap()
    cmask = nc.dram_tensor("cmask", [128, 128], BF16, kind="ExternalInput").ap()
    out = nc.dram_tensor("out", [S, D], F32, kind="ExternalOutput").ap()

    with tile.TileContext(nc) as tc:
        with ExitStack() as ctx:
            consts = ctx.enter_context(tc.tile_pool(name="consts", bufs=1))
            expp = ctx.enter_context(tc.tile_pool(name="expp", bufs=6))
            small = ctx.enter_context(tc.tile_pool(name="small", bufs=2))
            esp = ctx.enter_context(tc.tile_pool(name="esp", bufs=2))
            ostage = ctx.enter_context(tc.tile_pool(name="ostage", bufs=2))
            psc = ctx.enter_context(tc.tile_pool(name="psc", bufs=2, space="PSUM"))
            pctx = ctx.enter_context(tc.tile_pool(name="pctx", bufs=2, space="PSUM"))
            pp = ctx.enter_context(tc.tile_pool(name="pp", bufs=2, space="PSUM"))

            # ---- persistent SBUF tensors ----
            t_x = consts.tile([128, NKD, S], BF16, tag="x")
            t_w = consts.tile([128, NKD, 3 * HG], BF16, tag="w")
            t_bqk = consts.tile([128, NP, 2], F32, tag="bqk")
            t_bv = consts.tile([1, HG], F32, tag="bv")
            t_bvb = consts.tile([128, HG], F32, tag="bvb")
            t_ones64 = consts.tile([128, 64], BF16, tag="ones64")
            t_wo = consts.tile([128, NP, D], BF16, tag="wo")
            t_mask = consts.tile([128, 128], BF16, tag="mask")
            qT = consts.tile([128, NP, S], BF16, tag="qT")
            kT = consts.tile([128, NP, S], BF16, tag="kT")
            tv = consts.tile([128, NKT, 8 * 64], BF16, tag="tv")
            ctxT = consts.tile([128, NP, S], BF16, tag="ctxT")

            # input DMAs: interleave weights + x chunk 0 per contraction
            # tile, matching proj(0)'s kd-order consumption so the first
            # matmul starts after ~1 tile instead of the full 7MB.
            nc.sync.dma_start(t_bqk, bqk)
            nc.sync.dma_start(t_bv, bv)
            nc.sync.dma_start(t_mask, cmask)
            for kd in range(NKD):
                nc.sync.dma_start(t_w[:, kd, :], wqkv[:, kd, :])
                nc.sync.dma_start(t_x[:, kd, 0:QC], xt[:, kd, 0:QC])
            nc.sync.dma_start(t_wo, wo)
            for qc in range(1, NQC):
                nc.sync.dma_start(
                    t_x[:, :, qc * QC:(qc + 1) * QC], xt[:, :, qc * QC:(qc + 1) * QC]
                )
            # broadcast the v bias to all partitions once (keys dim)
            nc.gpsimd.partition_broadcast(t_bvb, t_bv)
            nc.vector.memset(t_ones64, 1.0)

            # HAM warmup: ~3.4us of garbage matmuls during the input DMA
            # fill so the PE clock is at 8/8 when the real stream starts.
            # Operands live in ctxT's last-written region (no early reader).
            ps_warm = pctx.tile([128, QC], F32, tag="c", name="ps_warm")
            for _ in range(16):
                nc.tensor.matmul(
                    ps_warm[0:64, :], ctxT[:, 3, 3 * QC:3 * QC + 64],
                    ctxT[:, 3, 3 * QC:4 * QC], start=True, stop=True,
                )

            # ---------- unit emitters ----------
            def proj_qk_unit(m, p, qc, pool, tag):
                ps = pool.tile([128, QC], F32, tag=tag, name="ps_p")
                c0 = p * HG + m * 128
                for kd in range(NKD):
                    nc.tensor.matmul(
                        ps, t_w[:, kd, c0:c0 + 128],
                        t_x[:, kd, qc * QC:(qc + 1) * QC],
                        start=(kd == 0), stop=(kd == NKD - 1),
                    )
                dst = (qT if p == 0 else kT)[:, m, qc * QC:(qc + 1) * QC]
                nc.vector.tensor_scalar_add(dst, ps, t_bqk[:, m, p:p + 1])

            def proj_v_unit(t, qc, pool, tag):
                kt = 4 * qc + t
                ps = pool.tile([128, QC], F32, tag=tag, name="ps_v")
                for kd in range(NKD):
                    nc.tensor.matmul(
                        ps, t_x[:, kd, kt * 128:(kt + 1) * 128],
                        t_w[:, kd, 2 * HG:3 * HG],
                        start=(kd == 0), stop=(kd == NKD - 1),
                    )
                dst = tv[:, kt, :].rearrange("p (h c) -> p h c", h=8)
                nc.vector.tensor_add(
                    dst, ps.rearrange("p (h c) -> p h c", h=8),
                    t_bvb.rearrange("p (h c) -> p h c", h=8),
                )

            def proj_units(qc, pool_alt=False):
                units = []
                for m in range(NP):
                    for p in range(2):
                        units.append((proj_qk_unit, m, p, qc))
                for t in range(4):
                    units.append((proj_v_unit, t, qc))
                out_units = []
                for i, u in enumerate(units):
                    fn, *args = u
                    if pool_alt and i % 2 == 1:
                        pool, tag = psc, "s"
                    else:
                        pool, tag = pp, "p"
                    out_units.append(lambda fn=fn, args=args, pool=pool, tag=tag:
                                     fn(*args, pool, tag))
                return out_units

            def attn_scores_unit(g, qc, kt, st):
                # diagonal tiles (o >= 1): queries j < 128*o in this chunk
                # are fully masked for this key tile — trim scores matmul,
                # exp, and the downstream ctx matmul to columns [128*o, QC).
                o = kt - 4 * qc
                tr = 128 * o if o > 0 else 0
                ps_s = psc.tile([128, 2 * QC], F32, tag="s", name="ps_s")
                # dependency-free ldweights ahead of the psum-gated matmuls
                # so the weight loads overlap the previous tile's streams
                nc.tensor.ldweights(
                    kT[0:64, g, kt * 128:(kt + 1) * 128], tile_position=(0, 0))
                nc.tensor.matmul(
                    ps_s[:, tr:QC],
                    kT[0:64, g, kt * 128:(kt + 1) * 128],
                    qT[0:64, g, qc * QC + tr:(qc + 1) * QC],
                    start=True, stop=True,
                )
                nc.tensor.ldweights(
                    kT[64:128, g, kt * 128:(kt + 1) * 128], tile_position=(64, 0))
                nc.tensor.matmul(
                    ps_s[:, QC + tr:2 * QC],
                    kT[64:128, g, kt * 128:(kt + 1) * 128],
                    qT[64:128, g, qc * QC + tr:(qc + 1) * QC],
                    start=True, stop=True, tile_position=(64, 0),
                )
                e = expp.tile([128, 2 * QC], BF16, tag="exp", name="t_e")
                if tr:
                    nc.scalar.activation(
                        e.rearrange("p (h w) -> p h w", h=2)[:, :, tr:QC],
                        ps_s.rearrange("p (h w) -> p h w", h=2)[:, :, tr:QC],
                        AF.Exp, scale=0.125,
                    )
                else:
                    nc.scalar.activation(e, ps_s, AF.Exp, scale=0.125)
                if o >= 0:
                    band = e.rearrange("p (h w) -> p h w", h=2)[:, :, tr:tr + 128]
                    nc.vector.tensor_mul(
                        band, band,
                        t_mask[:, None, :].broadcast_to([128, 2, 128]),
                    )
                st[kt] = e

            def attn_ctx_unit(g, qc, kt, st):
                # both heads' ctx matmuls (M=64 each) run concurrently on
                # disjoint PE column groups; denominators accumulate as
                # exp-sums on the vector engine instead of a V ones-column.
                nkt = 4 * qc + 4
                o = kt - 4 * qc
                tr = 128 * o if o > 0 else 0
                if kt == 0:
                    st["c"] = pctx.tile([128, QC], F32, tag="c", name="ps_c")
                    st["esum"] = esp.tile([128, 2 * QC], BF16, tag="esum",
                                          name="t_esum")
                e = st.pop(kt)
                c = st["c"]
                nc.tensor.ldweights(
                    tv[:, kt, 128 * g:128 * g + 64], tile_position=(0, 0))
                nc.tensor.matmul(
                    c[0:64, tr:QC], tv[:, kt, 128 * g:128 * g + 64],
                    e[:, tr:QC],
                    start=(kt == 0), stop=(kt == nkt - 1),
                    tile_position=(0, 0),
                )
                nc.tensor.ldweights(
                    tv[:, kt, 128 * g + 64:128 * g + 128], tile_position=(0, 64))
                nc.tensor.matmul(
                    c[64:128, tr:QC], tv[:, kt, 128 * g + 64:128 * g + 128],
                    e[:, QC + tr:2 * QC],
                    start=(kt == 0), stop=(kt == nkt - 1),
                    tile_position=(0, 64),
                )
                if kt == 0:
                    nc.vector.tensor_copy(st["esum"], e)
                else:
                    ev = e.rearrange("p (h w) -> p h w", h=2)[:, :, tr:QC]
                    sv = st["esum"].rearrange("p (h w) -> p h w", h=2)[:, :, tr:QC]
                    nc.vector.tensor_add(sv, sv, ev)

            def normalize_unit(g, qc, st):
                # denominators: two concurrent ones-matmuls column-sum the
                # esum halves; lhsT=ones[128,64] replicates each head's sums
                # across 64 partitions, landing in psum already aligned with
                # the ctx layout — one reciprocal + one multiply finish it.
                c = st.pop("c")
                esum = st.pop("esum")
                ps_d = pp.tile([128, QC], F32, tag="p", name="ps_d")
                nc.tensor.matmul(
                    ps_d[0:64, :], t_ones64, esum[:, 0:QC],
                    start=True, stop=True, tile_position=(0, 0),
                )
                nc.tensor.matmul(
                    ps_d[64:128, :], t_ones64, esum[:, QC:2 * QC],
                    start=True, stop=True, tile_position=(0, 64),
                )
                t_rec = small.tile([128, QC], F32, tag="rec", name="t_rec")
                nc.vector.reciprocal_approx_fast(t_rec, ps_d)
                nc.vector.tensor_mul(
                    ctxT[:, g, qc * QC:(qc + 1) * QC], c, t_rec,
                )

            def outproj_unit(qt):
                stg = ostage.tile([128, D], F32, tag="ost", name="stg")
                for ch in range(2):
                    ps_o = pp.tile([128, QC], F32, tag="p", name="ps_o")
                    for g in range(NP):
                        nc.tensor.matmul(
                            ps_o, ctxT[:, g, qt * 128:(qt + 1) * 128],
                            t_wo[:, g, ch * QC:(ch + 1) * QC],
                            start=(g == 0), stop=(g == NP - 1),
                        )
                    if ch == 0:
                        nc.vector.tensor_copy(stg[:, 0:QC], ps_o)
                    else:
                        nc.scalar.copy(stg[:, QC:2 * QC], ps_o)
                nc.sync.dma_start(out[qt * 128:(qt + 1) * 128, :], stg)

            def attn_units(qc):
                """Software-pipelined attention for all 4 pairs.

                Returns (kind, g, fn) tuples; kind 'c' marks ctx matmuls that
                wait on the scalar exp — the weave slots independent proj /
                out-proj work right before those to hide the latency.
                """
                units = []
                nkt = 4 * qc + 4
                for g in range(NP):
                    st = {}
                    for kt in range(nkt):
                        units.append(('s', g, lambda g=g, qc=qc, kt=kt, st=st:
                                      attn_scores_unit(g, qc, kt, st)))
                        if kt >= 1:
                            units.append(('c', g, lambda g=g, qc=qc, kt=kt - 1, st=st:
                                          attn_ctx_unit(g, qc, kt, st)))
                    units.append(('c', g, lambda g=g, qc=qc, st=st:
                                  attn_ctx_unit(g, qc, nkt - 1, st)))
                    units.append(('n', g, lambda g=g, qc=qc, st=st:
                                  normalize_unit(g, qc, st)))
                return units

            def outproj_units(qc):
                return [lambda qt=qt: outproj_unit(qt)
                        for qt in range(4 * qc, 4 * qc + 4)]

            def qk_unit_l(m, p, qc):
                return lambda: proj_qk_unit(m, p, qc, pp, "p")

            def v_unit_l(t, qc):
                return lambda: proj_v_unit(t, qc, pp, "p")

            # ---------- schedule ----------
            # Filler is placed by deadline so the scalar-paced late chunks
            # still have tensor work: q-projections for chunk qc+1 weave
            # into chunk qc; k/v-projections for chunk qc weave into chunk
            # qc itself ahead of their first (diagonal) consumer; ALL
            # out-projections for chunks 0-2 weave into chunk 3, which has
            # the most exp latency to hide.
            for u in proj_units(0, pool_alt=True):
                u()
            for qc in range(NQC):
                a = attn_units(qc)
                c_glob = [i for i, (k, g, f) in enumerate(a) if k == 'c']
                c_pair = [[i for i, (k, g, f) in enumerate(a)
                           if k == 'c' and g == gg] for gg in range(NP)]
                assign = {}

                def place(slot, unit):
                    assign.setdefault(slot, []).append(unit)

                if qc >= 1:
                    # k-unit for pair m: due before pair m's diagonal tiles.
                    for m in range(NP):
                        place(c_pair[m][0], qk_unit_l(m, 1, qc))
                    # v-unit t: due before pair 0's ctx of key tile 4qc+t.
                    for t in range(4):
                        place(c_pair[0][min(qc * (t + 1), 4 * qc + t - 1)],
                              v_unit_l(t, qc))
                free = []
                if qc + 1 < NQC:
                    free += [qk_unit_l(m, 0, qc + 1) for m in range(NP)]
                if qc == NQC - 1:
                    for oc in range(NQC - 1):
                        free += outproj_units(oc)
                for j, u in enumerate(free):
                    place(c_glob[(j * len(c_glob)) // len(free)], u)
                for i, (k, g, fn) in enumerate(a):
                    for u in assign.get(i, ()):
                        u()
                    fn()
            for u in outproj_units(NQC - 1):
                u()

    nc.compile()
    return nc


def _host_inputs(x, wq, bq, wk, bk, wv, bv, wo, bo):
    import ml_dtypes
    bf16 = ml_dtypes.bfloat16
    x = np.asarray(x, np.float32)
    wq, wk, wv, wo = (np.asarray(a, np.float32) for a in (wq, wk, wv, wo))
    bq, bk, bv_, bo = (np.asarray(a, np.float32) for a in (bq, bk, bv, bo))

    # single 128x128 causal triangle band: every diagonal key tile sees
    # the same local pattern keep[jj >= p] once trimmed to its band.
    p = np.arange(128)[:, None]
    jj = np.arange(128)[None, :]
    cmask = (jj >= p).astype(np.float32).astype(bf16)

    in_maps = []
    for c in range(NCORES):
        b, g = c // 2, c % 2
        hs = slice(g * HG, (g + 1) * HG)
        xt = np.ascontiguousarray(
            x[b].T.reshape(NKD, 128, S).transpose(1, 0, 2)).astype(bf16)
        wqkv = np.concatenate([wq[:, hs], wk[:, hs], wv[:, hs]], axis=1)
        wqkv = np.ascontiguousarray(
            wqkv.reshape(NKD, 128, 3 * HG).transpose(1, 0, 2)).astype(bf16)
        bqk = np.stack([bq[hs].reshape(NP, 128), bk[hs].reshape(NP, 128)],
                       axis=-1)  # [NP, 128, 2]
        bqk = np.ascontiguousarray(bqk.transpose(1, 0, 2))
        bvc = np.ascontiguousarray(bv_[hs][None, :])
        woc = np.ascontiguousarray(
            wo[hs, :].reshape(NP, 128, D).transpose(1, 0, 2)).astype(bf16)
        in_maps.append({
            "xt": xt, "wqkv": wqkv, "bqk": bqk, "bv": bvc,
            "wo": woc, "cmask": cmask,
        })
    return in_maps


def kernel(x, wq, bq, wk, bk, wv, bv, wo, bo, _trace=False, _tmpdir=None):
    if "nc" not in _CACHE:
        _CACHE["nc"] = _build()
    nc = _CACHE["nc"]
    in_maps = _host_inputs(x, wq, bq, wk, bk, wv, bv, wo, bo)
    res = bass_utils.run_bass_kernel_spmd(
        nc, in_maps, core_ids=list(range(NCORES)), trace=_trace, tmpdir=_tmpdir
    )
    _CACHE["last_results"] = res
    bo64 = np.asarray(bo, dtype=np.float64)[None, :]
    outs = []
    for b in range(B):
        acc = (res.results[2 * b]["out"].astype(np.float64)
               + res.results[2 * b + 1]["out"].astype(np.float64) + bo64)
        outs.append(acc.astype(np.float32))
    return np.stack(outs, axis=0)



# revision 25
# speedup vs baseline: 1.0843x; 1.0843x over previous
"""Causal multi-head attention on 8 Trainium2 NeuronCores.

v2: (batch x head-group) sharding — core c handles batch b=c//2 and heads
[8g:8g+8] (g=c%2), i.e. 4 row-packed head PAIRS per core. All matmul
operands are bf16 (fp32r streams at half the PE rate; bf16 runs 1 col/cycle
at 2.4GHz once the p-state ramps). PSUM accumulates in f32.

Per-core layout (partition dim first):
  t_x    [128, 8, 2048]  x_b^T tiled: [p, kd, seq], d = kd*128+p
  t_w    [128, 8, 1536]  wqkv tiles: [p, kd, q512|k512|v512]
  qT/kT  [128, 4, 2048]  pair m: partitions = m's 128 head dims
  tv     [128, 16, 512]  per key tile: 8 x v_head(64)
  scores [128, 1024]     psum, both heads of a pair (row-packed matmuls)
  ctx    [128, 512]      psum per pair: h0 rows 0:64, h1 rows 64:128
                         (col-tiled matmuls run both heads concurrently)
  esum   [128, 1024]     bf16 running exp-sums; softmax denominators via
                         gpsimd partition_all_reduce at pair end
  ctxT   [128, 4, 2048]  normalized context, bf16
  out    [2048, 1024]    f32 partial (host sums 2 cores + bo per batch)

V is computed directly in transposed orientation (x-tile stationary,
wv moving) so no on-chip transposes are needed. q/k biases fold into
the psum->SBUF copies (per-partition tensor_scalar_add). Diagonal key
tiles trim scores/exp/ctx to the unmasked query range.

Emission order software-pipelines the attention inner loop (scores kt+1
issued before ctx kt) and weaves projection units for q-chunk qc+1 into
the attention stream of q-chunk qc to keep the tensor engine dense.
"""

import numpy as np
from contextlib import ExitStack

import concourse.bass as bass
import concourse.mybir as mybir
import concourse.tile as tile
from concourse import bacc
from concourse import bass_utils
from concourse.bass import bass_isa

F32 = mybir.dt.float32
BF16 = mybir.dt.bfloat16
AF = mybir.ActivationFunctionType

B, S, D = 4, 2048, 1024
H, DH = 16, 64
NCORES = 8
HG = 512            # head dims per core (8 heads)
NP = 4              # head pairs per core
QC = 512            # q-chunk width
NQC = S // QC       # 4
NKD = D // 128      # 8 contraction tiles
NKT = S // 128      # 16 key tiles

_CACHE = {}


def _build():
    nc = bacc.Bacc("TRN2", target_bir_lowering=False, debug=False)
    xt = nc.dram_tensor("xt", [128, NKD, S], BF16, kind="ExternalInput").ap()
    wqkv = nc.dram_tensor("wqkv", [128, NKD, 3 * HG], BF16, kind="ExternalInput").ap()
    bqk = nc.dram_tensor("bqk", [128, NP, 2], F32, kind="ExternalInput").ap()
    bv = nc.dram_tensor("bv", [1, HG], F32, kind="ExternalInput").ap()
    wo = nc.dram_tensor("wo", [128, NP, D], BF16, kind="ExternalInput").ap()
    cmask = nc.dram_tensor("cmask", [128, 128], BF16, kind="ExternalInput").ap()
    out = nc.dram_tensor("out", [S, D], F32, kind="ExternalOutput").ap()

    with tile.TileContext(nc) as tc:
        with ExitStack() as ctx:
            consts = ctx.enter_context(tc.tile_pool(name="consts", bufs=1))
            expp = ctx.enter_context(tc.tile_pool(name="expp", bufs=6))
            small = ctx.enter_context(tc.tile_pool(name="small", bufs=2))
            esp = ctx.enter_context(tc.tile_pool(name="esp", bufs=2))
            ostage = ctx.enter_context(tc.tile_pool(name="ostage", bufs=2))
            psc = ctx.enter_context(tc.tile_pool(name="psc", bufs=2, space="PSUM"))
            pctx = ctx.enter_context(tc.tile_pool(name="pctx", bufs=2, space="PSUM"))
            pp = ctx.enter_context(tc.tile_pool(name="pp", bufs=2, space="PSUM"))

            # ---- persistent SBUF tensors ----
            t_x = consts.tile([128, NKD, S], BF16, tag="x")
            t_w = consts.tile([128, NKD, 3 * HG], BF16, tag="w")
            t_bqk = consts.tile([128, NP, 2], F32, tag="bqk")
            t_bv = consts.tile([1, HG], F32, tag="bv")
            t_bvb = consts.tile([128, HG], F32, tag="bvb")
            t_ones64 = consts.tile([128, 64], BF16, tag="ones64")
            t_wo = consts.tile([128, NP, D], BF16, tag="wo")
            t_mask = consts.tile([128, 128], BF16, tag="mask")
            qT = consts.tile([128, NP, S], BF16, tag="qT")
            kT = consts.tile([128, NP, S], BF16, tag="kT")
            tv = consts.tile([128, NKT, 8 * 64], BF16, tag="tv")
            ctxT = consts.tile([128, NP, S], BF16, tag="ctxT")

            # input DMAs: interleave weights + x chunk 0 per contraction
            # tile, matching proj(0)'s kd-order consumption so the first
            # matmul starts after ~1 tile instead of the full 7MB.
            nc.sync.dma_start(t_bqk, bqk)
            nc.sync.dma_start(t_bv, bv)
            nc.sync.dma_start(t_mask, cmask)
            for kd in range(NKD):
                nc.sync.dma_start(t_w[:, kd, :], wqkv[:, kd, :])
                nc.sync.dma_start(t_x[:, kd, 0:QC], xt[:, kd, 0:QC])
            nc.sync.dma_start(t_wo, wo)
            for qc in range(1, NQC):
                nc.sync.dma_start(
                    t_x[:, :, qc * QC:(qc + 1) * QC], xt[:, :, qc * QC:(qc + 1) * QC]
                )
            # broadcast the v bias to all partitions once (keys dim)
            nc.gpsimd.partition_broadcast(t_bvb, t_bv)
            nc.vector.memset(t_ones64, 1.0)

            # HAM warmup: ~3.4us of garbage matmuls during the input DMA
            # fill so the PE clock is at 8/8 when the real stream starts.
            # Operands live in ctxT's last-written region (no early reader).
            ps_warm = pctx.tile([128, QC], F32, tag="c", name="ps_warm")
            for _ in range(16):
                nc.tensor.matmul(
                    ps_warm[0:64, :], ctxT[:, 3, 3 * QC:3 * QC + 64],
                    ctxT[:, 3, 3 * QC:4 * QC], start=True, stop=True,
                )

            # ---------- unit emitters ----------
            def proj_qk_unit(m, p, qc, pool, tag):
                ps = pool.tile([128, QC], F32, tag=tag, name="ps_p")
                c0 = p * HG + m * 128
                for kd in range(NKD):
                    nc.tensor.matmul(
                        ps, t_w[:, kd, c0:c0 + 128],
                        t_x[:, kd, qc * QC:(qc + 1) * QC],
                        start=(kd == 0), stop=(kd == NKD - 1),
                    )
                dst = (qT if p == 0 else kT)[:, m, qc * QC:(qc + 1) * QC]
                nc.vector.tensor_scalar_add(dst, ps, t_bqk[:, m, p:p + 1])

            def proj_v_unit(t, qc, pool, tag):
                kt = 4 * qc + t
                ps = pool.tile([128, QC], F32, tag=tag, name="ps_v")
                for kd in range(NKD):
                    nc.tensor.matmul(
                        ps, t_x[:, kd, kt * 128:(kt + 1) * 128],
                        t_w[:, kd, 2 * HG:3 * HG],
                        start=(kd == 0), stop=(kd == NKD - 1),
                    )
                dst = tv[:, kt, :].rearrange("p (h c) -> p h c", h=8)
                nc.vector.tensor_add(
                    dst, ps.rearrange("p (h c) -> p h c", h=8),
                    t_bvb.rearrange("p (h c) -> p h c", h=8),
                )

            def proj_units(qc, pool_alt=False):
                units = []
                for m in range(NP):
                    for p in range(2):
                        units.append((proj_qk_unit, m, p, qc))
                for t in range(4):
                    units.append((proj_v_unit, t, qc))
                out_units = []
                for i, u in enumerate(units):
                    fn, *args = u
                    if pool_alt and i % 2 == 1:
                        pool, tag = psc, "s"
                    else:
                        pool, tag = pp, "p"
                    out_units.append(lambda fn=fn, args=args, pool=pool, tag=tag:
                                     fn(*args, pool, tag))
                return out_units

            def attn_scores_unit(g, qc, kt, st):
                # diagonal tiles (o >= 1): queries j < 128*o in this chunk
                # are fully masked for this key tile — trim scores matmul,
                # exp, and the downstream ctx matmul to columns [128*o, QC).
                o = kt - 4 * qc
                tr = 128 * o if o > 0 else 0
                ps_s = psc.tile([128, 2 * QC], F32, tag="s", name="ps_s")
                nc.tensor.matmul(
                    ps_s[:, tr:QC],
                    kT[0:64, g, kt * 128:(kt + 1) * 128],
                    qT[0:64, g, qc * QC + tr:(qc + 1) * QC],
                    start=True, stop=True,
                )
                nc.tensor.matmul(
                    ps_s[:, QC + tr:2 * QC],
                    kT[64:128, g, kt * 128:(kt + 1) * 128],
                    qT[64:128, g, qc * QC + tr:(qc + 1) * QC],
                    start=True, stop=True, tile_position=(64, 0),
                )
                e = expp.tile([128, 2 * QC], BF16, tag="exp", name="t_e")
                if tr:
                    nc.scalar.activation(
                        e.rearrange("p (h w) -> p h w", h=2)[:, :, tr:QC],
                        ps_s.rearrange("p (h w) -> p h w", h=2)[:, :, tr:QC],
                        AF.Exp, scale=0.125,
                    )
                else:
                    nc.scalar.activation(e, ps_s, AF.Exp, scale=0.125)
                if o >= 0:
                    band = e.rearrange("p (h w) -> p h w", h=2)[:, :, tr:tr + 128]
                    nc.vector.tensor_mul(
                        band, band,
                        t_mask[:, None, :].broadcast_to([128, 2, 128]),
                    )
                st[kt] = e

            def attn_ctx_unit(g, qc, kt, st):
                # both heads' ctx matmuls (M=64 each) run concurrently on
                # disjoint PE column groups; denominators accumulate as
                # exp-sums on the vector engine instead of a V ones-column.
                nkt = 4 * qc + 4
                o = kt - 4 * qc
                tr = 128 * o if o > 0 else 0
                if kt == 0:
                    st["c"] = pctx.tile([128, QC], F32, tag="c", name="ps_c")
                    st["esum"] = esp.tile([128, 2 * QC], BF16, tag="esum",
                                          name="t_esum")
                e = st.pop(kt)
                c = st["c"]
                nc.tensor.matmul(
                    c[0:64, tr:QC], tv[:, kt, 128 * g:128 * g + 64],
                    e[:, tr:QC],
                    start=(kt == 0), stop=(kt == nkt - 1),
                    tile_position=(0, 0),
                )
                nc.tensor.matmul(
                    c[64:128, tr:QC], tv[:, kt, 128 * g + 64:128 * g + 128],
                    e[:, QC + tr:2 * QC],
                    start=(kt == 0), stop=(kt == nkt - 1),
                    tile_position=(0, 64),
                )
                if kt == 0:
                    nc.vector.tensor_copy(st["esum"], e)
                else:
                    ev = e.rearrange("p (h w) -> p h w", h=2)[:, :, tr:QC]
                    sv = st["esum"].rearrange("p (h w) -> p h w", h=2)[:, :, tr:QC]
                    nc.vector.tensor_add(sv, sv, ev)

            def normalize_unit(g, qc, st):
                # denominators: two concurrent ones-matmuls column-sum the
                # esum halves; lhsT=ones[128,64] replicates each head's sums
                # across 64 partitions, landing in psum already aligned with
                # the ctx layout — one reciprocal + one multiply finish it.
                c = st.pop("c")
                esum = st.pop("esum")
                ps_d = pp.tile([128, QC], F32, tag="p", name="ps_d")
                nc.tensor.matmul(
                    ps_d[0:64, :], t_ones64, esum[:, 0:QC],
                    start=True, stop=True, tile_position=(0, 0),
                )
                nc.tensor.matmul(
                    ps_d[64:128, :], t_ones64, esum[:, QC:2 * QC],
                    start=True, stop=True, tile_position=(0, 64),
                )
                t_rec = small.tile([128, QC], F32, tag="rec", name="t_rec")
                nc.vector.reciprocal_approx_fast(t_rec, ps_d)
                nc.vector.tensor_mul(
                    ctxT[:, g, qc * QC:(qc + 1) * QC], c, t_rec,
                )

            def outproj_unit(qt):
                stg = ostage.tile([128, D], F32, tag="ost", name="stg")
                for ch in range(2):
                    ps_o = pp.tile([128, QC], F32, tag="p", name="ps_o")
                    for g in range(NP):
                        nc.tensor.matmul(
                            ps_o, ctxT[:, g, qt * 128:(qt + 1) * 128],
                            t_wo[:, g, ch * QC:(ch + 1) * QC],
                            start=(g == 0), stop=(g == NP - 1),
                        )
                    if ch == 0:
                        nc.vector.tensor_copy(stg[:, 0:QC], ps_o)
                    else:
                        nc.scalar.copy(stg[:, QC:2 * QC], ps_o)
                nc.sync.dma_start(out[qt * 128:(qt + 1) * 128, :], stg)

            def attn_units(qc):
                """Software-pipelined attention for all 4 pairs.

                Returns (kind, g, fn) tuples; kind 'c' marks ctx matmuls that
                wait on the scalar exp — the weave slots independent proj /
                out-proj work right before those to hide the latency.
                """
                units = []
                nkt = 4 * qc + 4
                for g in range(NP):
                    st = {}
                    for kt in range(nkt):
                        units.append(('s', g, lambda g=g, qc=qc, kt=kt, st=st:
                                      attn_scores_unit(g, qc, kt, st)))
                        if kt >= 1:
                            units.append(('c', g, lambda g=g, qc=qc, kt=kt - 1, st=st:
                                          attn_ctx_unit(g, qc, kt, st)))
                    units.append(('c', g, lambda g=g, qc=qc, st=st:
                                  attn_ctx_unit(g, qc, nkt - 1, st)))
                    units.append(('n', g, lambda g=g, qc=qc, st=st:
                                  normalize_unit(g, qc, st)))
                return units

            def outproj_units(qc):
                return [lambda qt=qt: outproj_unit(qt)
                        for qt in range(4 * qc, 4 * qc + 4)]

            def qk_unit_l(m, p, qc):
                return lambda: proj_qk_unit(m, p, qc, pp, "p")

            def v_unit_l(t, qc):
                return lambda: proj_v_unit(t, qc, pp, "p")

            # ---------- schedule ----------
            # Filler is placed by deadline so the scalar-paced late chunks
            # still have tensor work: only pair 0's chunk-0 projections run
            # before attention (exp starts ~10us earlier); the rest of
            # proj(0) weaves into attn(0) ahead of each pair's start.
            # q-projections for chunk qc+1 weave into chunk qc;
            # k/v-projections for chunk qc weave into chunk qc itself ahead
            # of their first (diagonal) consumer; out-projections go where
            # exp latency needs hiding most (chunk 2 and 3).
            proj_qk_unit(0, 0, 0, pp, "p")
            proj_qk_unit(0, 1, 0, psc, "s")
            proj_v_unit(0, 0, pp, "p")
            proj_v_unit(1, 0, psc, "s")
            for qc in range(NQC):
                a = attn_units(qc)
                c_glob = [i for i, (k, g, f) in enumerate(a) if k == 'c']
                c_pair = [[i for i, (k, g, f) in enumerate(a)
                           if k == 'c' and g == gg] for gg in range(NP)]
                assign = {}

                def place(slot, unit):
                    assign.setdefault(slot, []).append(unit)

                if qc == 0:
                    # rest of proj(0): v(2)/v(3) before pair 0 needs them;
                    # pair m's q/k units just before pair m starts.
                    place(c_pair[0][0], v_unit_l(2, 0))
                    place(c_pair[0][1], v_unit_l(3, 0))
                    for m in range(1, NP):
                        place(c_pair[m - 1][-1], qk_unit_l(m, 0, 0))
                        place(c_pair[m - 1][-1], qk_unit_l(m, 1, 0))
                else:
                    # k-unit for pair m: due before pair m's diagonal tiles.
                    for m in range(NP):
                        place(c_pair[m][0], qk_unit_l(m, 1, qc))
                    # v-unit t: due before pair 0's ctx of key tile 4qc+t.
                    for t in range(4):
                        place(c_pair[0][min(qc * (t + 1), 4 * qc + t - 1)],
                              v_unit_l(t, qc))
                free = []
                if qc + 1 < NQC:
                    free += [qk_unit_l(m, 0, qc + 1) for m in range(NP)]
                if qc == 2:
                    free += outproj_units(0)
                if qc == NQC - 1:
                    free += outproj_units(1) + outproj_units(2)
                for j, u in enumerate(free):
                    place(c_glob[(j * len(c_glob)) // len(free)], u)
                for i, (k, g, fn) in enumerate(a):
                    for u in assign.get(i, ()):
                        u()
                    fn()
            for u in outproj_units(NQC - 1):
                u()

    nc.compile()
    return nc


def _host_inputs(x, wq, bq, wk, bk, wv, bv, wo, bo):
    import ml_dtypes
    bf16 = ml_dtypes.bfloat16
    x = np.asarray(x, np.float32)
    wq, wk, wv, wo = (np.asarray(a, np.float32) for a in (wq, wk, wv, wo))
    bq, bk, bv_, bo = (np.asarray(a, np.float32) for a in (bq, bk, bv, bo))

    # single 128x128 causal triangle band: every diagonal key tile sees
    # the same local pattern keep[jj >= p] once trimmed to its band.
    p = np.arange(128)[:, None]
    jj = np.arange(128)[None, :]
    cmask = (jj >= p).astype(np.float32).astype(bf16)

    in_maps = []
    for c in range(NCORES):
        b, g = c // 2, c % 2
        hs = slice(g * HG, (g + 1) * HG)
        xt = np.ascontiguousarray(
            x[b].T.reshape(NKD, 128, S).transpose(1, 0, 2)).astype(bf16)
        wqkv = np.concatenate([wq[:, hs], wk[:, hs], wv[:, hs]], axis=1)
        wqkv = np.ascontiguousarray(
            wqkv.reshape(NKD, 128, 3 * HG).transpose(1, 0, 2)).astype(bf16)
        bqk = np.stack([bq[hs].reshape(NP, 128), bk[hs].reshape(NP, 128)],
                       axis=-1)  # [NP, 128, 2]
        bqk = np.ascontiguousarray(bqk.transpose(1, 0, 2))
        bvc = np.ascontiguousarray(bv_[hs][None, :])
        woc = np.ascontiguousarray(
            wo[hs, :].reshape(NP, 128, D).transpose(1, 0, 2)).astype(bf16)
        in_maps.append({
            "xt": xt, "wqkv": wqkv, "bqk": bqk, "bv": bvc,
            "wo": woc, "cmask": cmask,
        })
    return in_maps


def kernel(x, wq, bq, wk, bk, wv, bv, wo, bo, _trace=False, _tmpdir=None):
    if "nc" not in _CACHE:
        _CACHE["nc"] = _build()
    nc = _CACHE["nc"]
    in_maps = _host_inputs(x, wq, bq, wk, bk, wv, bv, wo, bo)
    res = bass_utils.run_bass_kernel_spmd(
        nc, in_maps, core_ids=list(range(NCORES)), trace=_trace, tmpdir=_tmpdir
    )
    _CACHE["last_results"] = res
    bo64 = np.asarray(bo, dtype=np.float64)[None, :]
    outs = []
    for b in range(B):
        acc = (res.results[2 * b]["out"].astype(np.float64)
               + res.results[2 * b + 1]["out"].astype(np.float64) + bo64)
        outs.append(acc.astype(np.float32))
    return np.stack(outs, axis=0)



# revision 26
# speedup vs baseline: 1.1598x; 1.0696x over previous
"""Causal multi-head attention on 8 Trainium2 NeuronCores.

v2: (batch x head-group) sharding — core c handles batch b=c//2 and heads
[8g:8g+8] (g=c%2), i.e. 4 row-packed head PAIRS per core. All matmul
operands are bf16 (fp32r streams at half the PE rate; bf16 runs 1 col/cycle
at 2.4GHz once the p-state ramps). PSUM accumulates in f32.

Per-core layout (partition dim first):
  t_x    [128, 8, 2048]  x_b^T tiled: [p, kd, seq], d = kd*128+p
  t_w    [128, 8, 1536]  wqkv tiles: [p, kd, q512|k512|v512]
  qT/kT  [128, 4, 2048]  pair m: partitions = m's 128 head dims
  tv     [128, 16, 512]  per key tile: 8 x v_head(64)
  scores [128, 1024]     psum, both heads of a pair (row-packed matmuls)
  ctx    [128, 512]      psum per pair: h0 rows 0:64, h1 rows 64:128
                         (col-tiled matmuls run both heads concurrently)
  esum   [128, 1024]     bf16 running exp-sums; softmax denominators via
                         gpsimd partition_all_reduce at pair end
  ctxT   [128, 4, 2048]  normalized context, bf16
  out    [2048, 1024]    f32 partial (host sums 2 cores + bo per batch)

V is computed directly in transposed orientation (x-tile stationary,
wv moving) so no on-chip transposes are needed. q/k biases fold into
the psum->SBUF copies (per-partition tensor_scalar_add). Diagonal key
tiles trim scores/exp/ctx to the unmasked query range.

Emission order software-pipelines the attention inner loop (scores kt+1
issued before ctx kt) and weaves projection units for q-chunk qc+1 into
the attention stream of q-chunk qc to keep the tensor engine dense.
"""

import numpy as np
from contextlib import ExitStack

import concourse.bass as bass
import concourse.mybir as mybir
import concourse.tile as tile
from concourse import bacc
from concourse import bass_utils
from concourse.bass import bass_isa

F32 = mybir.dt.float32
BF16 = mybir.dt.bfloat16
AF = mybir.ActivationFunctionType

B, S, D = 4, 2048, 1024
H, DH = 16, 64
NCORES = 8
HG = 512            # head dims per core (8 heads)
NP = 4              # head pairs per core
QC = 512            # q-chunk width
NQC = S // QC       # 4
NKD = D // 128      # 8 contraction tiles
NKT = S // 128      # 16 key tiles

_CACHE = {}


def _build():
    nc = bacc.Bacc("TRN2", target_bir_lowering=False, debug=False)
    xt = nc.dram_tensor("xt", [128, NKD, S], BF16, kind="ExternalInput").ap()
    wqkv = nc.dram_tensor("wqkv", [128, NKD, 3 * HG], BF16, kind="ExternalInput").ap()
    bqk = nc.dram_tensor("bqk", [128, NP, 2], F32, kind="ExternalInput").ap()
    bv = nc.dram_tensor("bv", [1, HG], F32, kind="ExternalInput").ap()
    wo = nc.dram_tensor("wo", [128, NP, D], BF16, kind="ExternalInput").ap()
    cmask = nc.dram_tensor("cmask", [128, 128], BF16, kind="ExternalInput").ap()
    out = nc.dram_tensor("out", [S, D], F32, kind="ExternalOutput").ap()

    with tile.TileContext(nc) as tc:
        with ExitStack() as ctx:
            consts = ctx.enter_context(tc.tile_pool(name="consts", bufs=1))
            expp = ctx.enter_context(tc.tile_pool(name="expp", bufs=6))
            small = ctx.enter_context(tc.tile_pool(name="small", bufs=2))
            esp = ctx.enter_context(tc.tile_pool(name="esp", bufs=2))
            ostage = ctx.enter_context(tc.tile_pool(name="ostage", bufs=2))
            psc = ctx.enter_context(tc.tile_pool(name="psc", bufs=2, space="PSUM"))
            pctx = ctx.enter_context(tc.tile_pool(name="pctx", bufs=2, space="PSUM"))
            pp = ctx.enter_context(tc.tile_pool(name="pp", bufs=2, space="PSUM"))

            # ---- persistent SBUF tensors ----
            t_x = consts.tile([128, NKD, S], BF16, tag="x")
            t_w = consts.tile([128, NKD, 3 * HG], BF16, tag="w")
            t_bqk = consts.tile([128, NP, 2], F32, tag="bqk")
            t_bv = consts.tile([1, HG], F32, tag="bv")
            t_bvb = consts.tile([128, HG], F32, tag="bvb")
            t_ones64 = consts.tile([128, 64], BF16, tag="ones64")
            t_wo = consts.tile([128, NP, D], BF16, tag="wo")
            t_mask = consts.tile([128, 128], BF16, tag="mask")
            qT = consts.tile([128, NP, S], BF16, tag="qT")
            kT = consts.tile([128, NP, S], BF16, tag="kT")
            tv = consts.tile([128, NKT, 8 * 64], BF16, tag="tv")
            ctxT = consts.tile([128, NP, S], BF16, tag="ctxT")

            # input DMAs: interleave weights + x chunk 0 per contraction
            # tile, matching proj(0)'s kd-order consumption so the first
            # matmul starts after ~1 tile instead of the full 7MB.
            nc.sync.dma_start(t_bqk, bqk)
            nc.sync.dma_start(t_bv, bv)
            nc.sync.dma_start(t_mask, cmask)
            for kd in range(NKD):
                nc.sync.dma_start(t_w[:, kd, :], wqkv[:, kd, :])
                nc.sync.dma_start(t_x[:, kd, 0:QC], xt[:, kd, 0:QC])
            nc.sync.dma_start(t_wo, wo)
            for qc in range(1, NQC):
                nc.sync.dma_start(
                    t_x[:, :, qc * QC:(qc + 1) * QC], xt[:, :, qc * QC:(qc + 1) * QC]
                )
            # broadcast the v bias to all partitions once (keys dim)
            nc.gpsimd.partition_broadcast(t_bvb, t_bv)
            nc.vector.memset(t_ones64, 1.0)

            # HAM warmup: ~3.4us of garbage matmuls during the input DMA
            # fill so the PE clock is at 8/8 when the real stream starts.
            # Operands live in ctxT's last-written region (no early reader).
            ps_warm = pctx.tile([128, QC], F32, tag="c", name="ps_warm")
            for _ in range(16):
                nc.tensor.matmul(
                    ps_warm[0:64, :], ctxT[:, 3, 3 * QC:3 * QC + 64],
                    ctxT[:, 3, 3 * QC:4 * QC], start=True, stop=True,
                )

            # ---------- unit emitters ----------
            def proj_qk_unit(m, p, qc, pool, tag):
                ps = pool.tile([128, QC], F32, tag=tag, name="ps_p")
                c0 = p * HG + m * 128
                for kd in range(NKD):
                    nc.tensor.matmul(
                        ps, t_w[:, kd, c0:c0 + 128],
                        t_x[:, kd, qc * QC:(qc + 1) * QC],
                        start=(kd == 0), stop=(kd == NKD - 1),
                    )
                dst = (qT if p == 0 else kT)[:, m, qc * QC:(qc + 1) * QC]
                nc.vector.tensor_scalar_add(dst, ps, t_bqk[:, m, p:p + 1])

            def proj_v_unit(t, qc, pool, tag):
                kt = 4 * qc + t
                ps = pool.tile([128, QC], F32, tag=tag, name="ps_v")
                for kd in range(NKD):
                    nc.tensor.matmul(
                        ps, t_x[:, kd, kt * 128:(kt + 1) * 128],
                        t_w[:, kd, 2 * HG:3 * HG],
                        start=(kd == 0), stop=(kd == NKD - 1),
                    )
                dst = tv[:, kt, :].rearrange("p (h c) -> p h c", h=8)
                nc.vector.tensor_add(
                    dst, ps.rearrange("p (h c) -> p h c", h=8),
                    t_bvb.rearrange("p (h c) -> p h c", h=8),
                )

            def proj_units(qc, pool_alt=False):
                units = []
                for m in range(NP):
                    for p in range(2):
                        units.append((proj_qk_unit, m, p, qc))
                for t in range(4):
                    units.append((proj_v_unit, t, qc))
                out_units = []
                for i, u in enumerate(units):
                    fn, *args = u
                    if pool_alt and i % 2 == 1:
                        pool, tag = psc, "s"
                    else:
                        pool, tag = pp, "p"
                    out_units.append(lambda fn=fn, args=args, pool=pool, tag=tag:
                                     fn(*args, pool, tag))
                return out_units

            def attn_scores_unit(g, qc, kt, st):
                # diagonal tiles (o >= 1): queries j < 128*o in this chunk
                # are fully masked for this key tile — trim scores matmul,
                # exp, and the downstream ctx matmul to columns [128*o, QC).
                o = kt - 4 * qc
                tr = 128 * o if o > 0 else 0
                ps_s = psc.tile([128, 2 * QC], F32, tag="s", name="ps_s")
                nc.tensor.matmul(
                    ps_s[:, tr:QC],
                    kT[0:64, g, kt * 128:(kt + 1) * 128],
                    qT[0:64, g, qc * QC + tr:(qc + 1) * QC],
                    start=True, stop=True,
                )
                nc.tensor.matmul(
                    ps_s[:, QC + tr:2 * QC],
                    kT[64:128, g, kt * 128:(kt + 1) * 128],
                    qT[64:128, g, qc * QC + tr:(qc + 1) * QC],
                    start=True, stop=True, tile_position=(64, 0),
                )
                e = expp.tile([128, 2 * QC], BF16, tag="exp", name="t_e")
                if tr:
                    nc.scalar.activation(
                        e.rearrange("p (h w) -> p h w", h=2)[:, :, tr:QC],
                        ps_s.rearrange("p (h w) -> p h w", h=2)[:, :, tr:QC],
                        AF.Exp, scale=0.125,
                    )
                else:
                    nc.scalar.activation(e, ps_s, AF.Exp, scale=0.125)
                if o >= 0:
                    band = e.rearrange("p (h w) -> p h w", h=2)[:, :, tr:tr + 128]
                    nc.vector.tensor_mul(
                        band, band,
                        t_mask[:, None, :].broadcast_to([128, 2, 128]),
                    )
                st[kt] = e

            def attn_ctx_unit(g, qc, kt, st):
                # both heads' ctx matmuls (M=64 each) run concurrently on
                # disjoint PE column groups; denominators accumulate as
                # exp-sums on the vector engine instead of a V ones-column.
                nkt = 4 * qc + 4
                o = kt - 4 * qc
                tr = 128 * o if o > 0 else 0
                if kt == 0:
                    st["c"] = pctx.tile([128, QC], F32, tag="c", name="ps_c")
                    st["esum"] = esp.tile([128, 2 * QC], BF16, tag="esum",
                                          name="t_esum")
                e = st.pop(kt)
                c = st["c"]
                nc.tensor.matmul(
                    c[0:64, tr:QC], tv[:, kt, 128 * g:128 * g + 64],
                    e[:, tr:QC],
                    start=(kt == 0), stop=(kt == nkt - 1),
                    tile_position=(0, 0),
                )
                nc.tensor.matmul(
                    c[64:128, tr:QC], tv[:, kt, 128 * g + 64:128 * g + 128],
                    e[:, QC + tr:2 * QC],
                    start=(kt == 0), stop=(kt == nkt - 1),
                    tile_position=(0, 64),
                )
                if kt == 0:
                    nc.vector.tensor_copy(st["esum"], e)
                else:
                    ev = e.rearrange("p (h w) -> p h w", h=2)[:, :, tr:QC]
                    sv = st["esum"].rearrange("p (h w) -> p h w", h=2)[:, :, tr:QC]
                    nc.vector.tensor_add(sv, sv, ev)

            def normalize_unit(g, qc, st):
                # denominators: two concurrent ones-matmuls column-sum the
                # esum halves; lhsT=ones[128,64] replicates each head's sums
                # across 64 partitions, landing in psum already aligned with
                # the ctx layout — one reciprocal + one multiply finish it.
                c = st.pop("c")
                esum = st.pop("esum")
                ps_d = pp.tile([128, QC], F32, tag="p", name="ps_d")
                nc.tensor.matmul(
                    ps_d[0:64, :], t_ones64, esum[:, 0:QC],
                    start=True, stop=True, tile_position=(0, 0),
                )
                nc.tensor.matmul(
                    ps_d[64:128, :], t_ones64, esum[:, QC:2 * QC],
                    start=True, stop=True, tile_position=(0, 64),
                )
                t_rec = small.tile([128, QC], F32, tag="rec", name="t_rec")
                nc.vector.reciprocal_approx_fast(t_rec, ps_d)
                nc.vector.tensor_mul(
                    ctxT[:, g, qc * QC:(qc + 1) * QC], c, t_rec,
                )

            def outproj_unit(qt):
                stg = ostage.tile([128, D], F32, tag="ost", name="stg")
                for ch in range(2):
                    ps_o = pp.tile([128, QC], F32, tag="p", name="ps_o")
                    for g in range(NP):
                        nc.tensor.matmul(
                            ps_o, ctxT[:, g, qt * 128:(qt + 1) * 128],
                            t_wo[:, g, ch * QC:(ch + 1) * QC],
                            start=(g == 0), stop=(g == NP - 1),
                        )
                    if ch == 0:
                        nc.vector.tensor_copy(stg[:, 0:QC], ps_o)
                    else:
                        nc.scalar.copy(stg[:, QC:2 * QC], ps_o)
                nc.sync.dma_start(out[qt * 128:(qt + 1) * 128, :], stg)

            def attn_units(qc):
                """Software-pipelined attention for all 4 pairs.

                Returns (kind, g, fn) tuples; kind 'c' marks ctx matmuls that
                wait on the scalar exp — the weave slots independent proj /
                out-proj work right before those to hide the latency.
                """
                units = []
                nkt = 4 * qc + 4

                def s_u(g, kt, st):
                    units.append(('s', g, lambda: attn_scores_unit(g, qc, kt, st)))

                def c_u(g, kt, st):
                    units.append(('c', g, lambda: attn_ctx_unit(g, qc, kt, st)))

                # batch-2 pipeline: two score pairs, then two ctx pairs.
                # within a pure scores (ctx) run, consecutive weight loads
                # target disjoint PE row (column) halves and overlap the
                # in-flight stream; only the batch transitions pay the
                # exposed load + drain.
                for g in range(NP):
                    st = {}
                    for j in range(nkt // 2):
                        s_u(g, 2 * j, st)
                        s_u(g, 2 * j + 1, st)
                        if j > 0:
                            c_u(g, 2 * j - 2, st)
                            c_u(g, 2 * j - 1, st)
                    c_u(g, nkt - 2, st)
                    c_u(g, nkt - 1, st)
                    units.append(('n', g, lambda g=g, st=st:
                                  normalize_unit(g, qc, st)))
                return units

            def outproj_units(qc):
                return [lambda qt=qt: outproj_unit(qt)
                        for qt in range(4 * qc, 4 * qc + 4)]

            def qk_unit_l(m, p, qc):
                return lambda: proj_qk_unit(m, p, qc, pp, "p")

            def v_unit_l(t, qc):
                return lambda: proj_v_unit(t, qc, pp, "p")

            # ---------- schedule ----------
            # Filler is placed by deadline so the scalar-paced late chunks
            # still have tensor work: only pair 0's chunk-0 projections run
            # before attention (exp starts ~10us earlier); the rest of
            # proj(0) weaves into attn(0) ahead of each pair's start.
            # q-projections for chunk qc+1 weave into chunk qc;
            # k/v-projections for chunk qc weave into chunk qc itself ahead
            # of their first (diagonal) consumer; out-projections go where
            # exp latency needs hiding most (chunk 2 and 3).
            proj_qk_unit(0, 0, 0, pp, "p")
            proj_qk_unit(0, 1, 0, psc, "s")
            proj_v_unit(0, 0, pp, "p")
            proj_v_unit(1, 0, psc, "s")
            for qc in range(NQC):
                a = attn_units(qc)
                c_glob = [i for i, (k, g, f) in enumerate(a) if k == 'c']
                c_pair = [[i for i, (k, g, f) in enumerate(a)
                           if k == 'c' and g == gg] for gg in range(NP)]
                assign = {}

                def place(slot, unit):
                    assign.setdefault(slot, []).append(unit)

                if qc == 0:
                    # rest of proj(0): v(2)/v(3) before pair 0 needs them;
                    # pair m's q/k units just before pair m starts.
                    place(c_pair[0][0], v_unit_l(2, 0))
                    place(c_pair[0][1], v_unit_l(3, 0))
                    for m in range(1, NP):
                        place(c_pair[m - 1][-1], qk_unit_l(m, 0, 0))
                        place(c_pair[m - 1][-1], qk_unit_l(m, 1, 0))
                else:
                    # k-unit for pair m: due before pair m's diagonal tiles.
                    for m in range(NP):
                        place(c_pair[m][0], qk_unit_l(m, 1, qc))
                    # v-unit t: due before pair 0's ctx of key tile 4qc+t.
                    for t in range(4):
                        place(c_pair[0][min(qc * (t + 1), 4 * qc + t - 1)],
                              v_unit_l(t, qc))
                free = []
                if qc + 1 < NQC:
                    free += [qk_unit_l(m, 0, qc + 1) for m in range(NP)]
                if qc == 2:
                    free += outproj_units(0)
                if qc == NQC - 1:
                    free += outproj_units(1) + outproj_units(2)
                for j, u in enumerate(free):
                    place(c_glob[(j * len(c_glob)) // len(free)], u)
                for i, (k, g, fn) in enumerate(a):
                    for u in assign.get(i, ()):
                        u()
                    fn()
            for u in outproj_units(NQC - 1):
                u()

    nc.compile()
    return nc


def _host_inputs(x, wq, bq, wk, bk, wv, bv, wo, bo):
    import ml_dtypes
    bf16 = ml_dtypes.bfloat16
    x = np.asarray(x, np.float32)
    wq, wk, wv, wo = (np.asarray(a, np.float32) for a in (wq, wk, wv, wo))
    bq, bk, bv_, bo = (np.asarray(a, np.float32) for a in (bq, bk, bv, bo))

    # single 128x128 causal triangle band: every diagonal key tile sees
    # the same local pattern keep[jj >= p] once trimmed to its band.
    p = np.arange(128)[:, None]
    jj = np.arange(128)[None, :]
    cmask = (jj >= p).astype(np.float32).astype(bf16)

    in_maps = []
    for c in range(NCORES):
        b, g = c // 2, c % 2
        hs = slice(g * HG, (g + 1) * HG)
        xt = np.ascontiguousarray(
            x[b].T.reshape(NKD, 128, S).transpose(1, 0, 2)).astype(bf16)
        wqkv = np.concatenate([wq[:, hs], wk[:, hs], wv[:, hs]], axis=1)
        wqkv = np.ascontiguousarray(
            wqkv.reshape(NKD, 128, 3 * HG).transpose(1, 0, 2)).astype(bf16)
        bqk = np.stack([bq[hs].reshape(NP, 128), bk[hs].reshape(NP, 128)],
                       axis=-1)  # [NP, 128, 2]
        bqk = np.ascontiguousarray(bqk.transpose(1, 0, 2))
        bvc = np.ascontiguousarray(bv_[hs][None, :])
        woc = np.ascontiguousarray(
            wo[hs, :].reshape(NP, 128, D).transpose(1, 0, 2)).astype(bf16)
        in_maps.append({
            "xt": xt, "wqkv": wqkv, "bqk": bqk, "bv": bvc,
            "wo": woc, "cmask": cmask,
        })
    return in_maps


def kernel(x, wq, bq, wk, bk, wv, bv, wo, bo, _trace=False, _tmpdir=None):
    if "nc" not in _CACHE:
        _CACHE["nc"] = _build()
    nc = _CACHE["nc"]
    in_maps = _host_inputs(x, wq, bq, wk, bk, wv, bv, wo, bo)
    res = bass_utils.run_bass_kernel_spmd(
        nc, in_maps, core_ids=list(range(NCORES)), trace=_trace, tmpdir=_tmpdir
    )
    _CACHE["last_results"] = res
    bo64 = np.asarray(bo, dtype=np.float64)[None, :]
    outs = []
    for b in range(B):
        acc = (res.results[2 * b]["out"].astype(np.float64)
               + res.results[2 * b + 1]["out"].astype(np.float64) + bo64)
        outs.append(acc.astype(np.float32))
    return np.stack(outs, axis=0)

